# revision 1
# baseline (speedup 1.0000x reference)
"""HolE scorer kernel for 8 Trainium2 NeuronCores (Bass/Tile).

Computation (reference):
    a = x @ W_e.T; b = y @ W_e.T; rr = r @ W_r.T          # (B, d)
    corr = irfft(rfft(a) * conj(rfft(b))) / d             # circular correlation
    out = sigmoid(sum(rr * corr, axis=1))                 # (B, 1)

Strategy:
  - Tensor-parallel over entities for the two big GEMMs: core c holds
    entity columns [c*12500, (c+1)*12500) of x, y, W_e (padded to 12544 =
    98*128), computes partial a.T/b.T (d-major), ReduceScatter(add) over
    the 8 cores hands core c the fully-summed 128-batch-row slice.
  - Tail per core (128 batch rows): rr.T GEMM, rfft via DFT-basis matmuls,
    and the irfft+rowwise-dot folded into a frequency-domain weighted dot
    (Parseval):  score_i = (1/d^2) sum_f w_f (Rr*Pr + Ri*Pi)[i,f],
    with P = A * conj(B), w = [1, 2, ..., 2, 1].  The w/d^2 factor is
    folded into the DFT basis used for R, so score = reduce_sum(R' . P).
  - All matmul inputs in bf16 (fp32 PSUM accumulation): validated max rel
    err ~2e-3 on the final sigmoid output.
"""

import numpy as np
import ml_dtypes

import concourse.bass as bass
import concourse.tile as tile
from concourse import bacc, mybir
from concourse.alu_op_type import AluOpType
from concourse.bass_utils import run_bass_kernel_spmd

# Problem shapes (hardcoded per contract)
B = 1024            # batch
D = 512             # num_dim
E = 100000          # num_entities
R = 1000            # num_relations
NCORES = 8

E_SH = E // NCORES          # 12500 entities per core
KC = 98                     # k-chunks of 128 after padding (98*128 = 12544)
E_PAD = KC * 128            # 12544
KG = 7                      # k-groups
KJ = KC // KG               # 14 chunks per group
RC = 8                      # relation k-chunks (1000 -> 1024)
R_PAD = RC * 128
NF = D // 2 + 1             # 257 rfft bins
B_SH = B // NCORES          # 128 batch rows per core

BF16 = mybir.dt.bfloat16
F32 = mybir.dt.float32

_cached = {}


def _dft_bases():
    d = D
    dd = np.arange(d, dtype=np.float64)[:, None]
    ff = np.arange(NF, dtype=np.float64)[None, :]
    ang = 2.0 * np.pi * dd * ff / d
    fr = np.cos(ang)
    fi = -np.sin(ang)
    f_ab = np.concatenate([fr, fi], axis=1)              # (512, 514)
    w = np.full(NF, 2.0); w[0] = 1.0; w[-1] = 1.0
    scale = w / (d * d)
    f_r = np.concatenate([fr * scale, fi * scale], axis=1)
    return (f_ab.astype(ml_dtypes.bfloat16), f_r.astype(ml_dtypes.bfloat16))


def _build_program():
    nc = bacc.Bacc("TRN2", target_bir_lowering=False, debug=False,
                   num_devices=NCORES)

    xT_d = nc.dram_tensor("xT", (E_PAD, B), BF16, kind="ExternalInput")
    yT_d = nc.dram_tensor("yT", (E_PAD, B), BF16, kind="ExternalInput")
    weT_d = nc.dram_tensor("weT", (E_PAD, D), BF16, kind="ExternalInput")
    rT_d = nc.dram_tensor("rT", (R_PAD, B_SH), BF16, kind="ExternalInput")
    wrT_d = nc.dram_tensor("wrT", (R_PAD, D), BF16, kind="ExternalInput")
    fab_d = nc.dram_tensor("fab", (D, 2 * NF), BF16, kind="ExternalInput")
    fr_d = nc.dram_tensor("fr", (D, 2 * NF), BF16, kind="ExternalInput")
    out_d = nc.dram_tensor("out", (B_SH, 1), F32, kind="ExternalOutput")

    # (core, mat, dim, batch) staging for the reduce-scatters
    stage_a = nc.dram_tensor("stage_a", (NCORES, D, B_SH), F32)
    stage_b = nc.dram_tensor("stage_b", (NCORES, D, B_SH), F32)
    rs_a = nc.dram_tensor("rs_a", (D, B_SH), F32)
    rs_b = nc.dram_tensor("rs_b", (D, B_SH), F32)
    groups = [list(range(NCORES))]

    with tile.TileContext(nc) as tc:
        with (
            tc.tile_pool(name="weights", bufs=1) as wpool,
            tc.tile_pool(name="stream", bufs=2) as spool,
            tc.tile_pool(name="copies", bufs=4) as cpool,
            tc.tile_pool(name="tail", bufs=1) as tpool,
        ):
            # ---- resident weights / small tensors ----
            we_tiles = []
            for g in range(KG):
                wt = wpool.tile([128, KJ, D], BF16, tag=f"we{g}", name=f"we{g}")
                nc.sync.dma_start(
                    wt[:],
                    weT_d[g * KJ * 128:(g + 1) * KJ * 128, :]
                    .rearrange("(j p) q -> p j q", p=128))
                we_tiles.append(wt)

            wr_t = wpool.tile([128, RC, D], BF16, tag="wr", name="wr")
            nc.sync.dma_start(
                wr_t[:], wrT_d[:].rearrange("(j p) q -> p j q", p=128))
            r_t = wpool.tile([128, RC, B_SH], BF16, tag="r", name="r")
            nc.sync.dma_start(
                r_t[:], rT_d[:].rearrange("(j p) q -> p j q", p=128))
            fab_t = wpool.tile([128, 4, 2 * NF], BF16, tag="fab", name="fab")
            nc.sync.dma_start(
                fab_t[:], fab_d[:].rearrange("(j p) q -> p j q", p=128))
            fr_t = wpool.tile([128, 4, 2 * NF], BF16, tag="frq", name="frq")
            nc.sync.dma_start(
                fr_t[:], fr_d[:].rearrange("(j p) q -> p j q", p=128))

            # ---- rr.T GEMM (independent of the big passes; 1 PSUM bank) ----
            with tc.tile_pool(name="psum_rr", bufs=1, space="PSUM") as prr:
                ps_rr = prr.tile([128, 4, B_SH], F32, name="ps_rr")
                for m in range(4):
                    for j in range(RC):
                        nc.tensor.matmul(
                            ps_rr[:, m, :],
                            wr_t[:, j, m * 128:(m + 1) * 128],
                            r_t[:, j, :],
                            start=(j == 0), stop=(j == RC - 1))
                rr_b = tpool.tile([128, 4, B_SH], BF16, name="rr_b")
                nc.vector.tensor_copy(rr_b[:], ps_rr[:])

            # ---- main GEMMs: a.T and b.T partials, d-major ----
            with tc.tile_pool(name="psum_main", bufs=8, space="PSUM") as pmain:
                for mi, (mat_d, stage) in enumerate(
                        [(xT_d, stage_a), (yT_d, stage_b)]):
                    for n in range(B // 512):
                        accs = [
                            pmain.tile([128, 512], F32, tag="acc",
                                       name=f"acc{mi}{n}{m}")
                            for m in range(4)
                        ]
                        for g in range(KG):
                            xt = spool.tile([128, KJ, 512], BF16, tag="xs",
                                            name=f"xs{mi}{n}{g}")
                            nc.sync.dma_start(
                                xt[:],
                                mat_d[g * KJ * 128:(g + 1) * KJ * 128,
                                      n * 512:(n + 1) * 512]
                                .rearrange("(j p) q -> p j q", p=128))
                            for j in range(KJ):
                                k = g * KJ + j
                                for m in range(4):
                                    nc.tensor.matmul(
                                        accs[m][:],
                                        we_tiles[g][:, j, m * 128:(m + 1) * 128],
                                        xt[:, j, :],
                                        start=(k == 0), stop=(k == KC - 1))
                        for m in range(4):
                            sb = cpool.tile([128, 512], F32, tag="cp",
                                            name=f"cp{mi}{n}{m}")
                            nc.vector.tensor_copy(sb[:], accs[m][:])
                            dst = (stage[4 * n:4 * n + 4,
                                         m * 128:(m + 1) * 128, :]
                                   .rearrange("c d q -> d c q"))
                            nc.sync.dma_start(
                                dst, sb.rearrange("d (c q) -> d c q", c=4))
                    # fire the reduce-scatter as soon as this matrix is done
                    rs_out = rs_a if mi == 0 else rs_b
                    nc.gpsimd.collective_compute(
                        "ReduceScatter", AluOpType.add,
                        replica_groups=groups,
                        ins=[stage[:].opt()], outs=[rs_out[:].opt()])

            # ---- tail: load reduced a.T/b.T slices, rfft matmuls, score ----
            aT = tpool.tile([128, 4, B_SH], F32, name="aT")
            nc.sync.dma_start(
                aT[:], rs_a[:].rearrange("(mc p) q -> p mc q", p=128))
            bT = tpool.tile([128, 4, B_SH], F32, name="bT")
            nc.sync.dma_start(
                bT[:], rs_b[:].rearrange("(mc p) q -> p mc q", p=128))
            aT_b = tpool.tile([128, 4, B_SH], BF16, name="aT_b")
            nc.vector.tensor_copy(aT_b[:], aT[:])
            bT_b = tpool.tile([128, 4, B_SH], BF16, name="bT_b")
            nc.vector.tensor_copy(bT_b[:], bT[:])

            with tc.tile_pool(name="psum_fft", bufs=1, space="PSUM") as pfft:
                def rfft_mm(src_b, basis, nm):
                    psr = pfft.tile([B_SH, NF], F32, tag=f"{nm}r", name=f"{nm}r")
                    psi = pfft.tile([B_SH, NF], F32, tag=f"{nm}i", name=f"{nm}i")
                    for k in range(4):
                        nc.tensor.matmul(psr[:], src_b[:, k, :],
                                         basis[:, k, 0:NF],
                                         start=(k == 0), stop=(k == 3))
                    for k in range(4):
                        nc.tensor.matmul(psi[:], src_b[:, k, :],
                                         basis[:, k, NF:2 * NF],
                                         start=(k == 0), stop=(k == 3))
                    return psr, psi

                ps_ar, ps_ai = rfft_mm(aT_b, fab_t, "a")
                ps_br, ps_bi = rfft_mm(bT_b, fab_t, "b")
                ps_rr_f, ps_ri_f = rfft_mm(rr_b, fr_t, "q")

                s_ar = tpool.tile([B_SH, NF], F32, name="s_ar")
                nc.vector.tensor_copy(s_ar[:], ps_ar[:])
                s_ai = tpool.tile([B_SH, NF], F32, name="s_ai")
                nc.vector.tensor_copy(s_ai[:], ps_ai[:])

                # P = A * conj(B): Pr = ArBr + AiBi ; Pi = AiBr - ArBi
                pr = tpool.tile([B_SH, NF], F32, name="pr")
                nc.vector.tensor_tensor(pr[:], ps_br[:], s_ar[:], AluOpType.mult)
                t1 = tpool.tile([B_SH, NF], F32, name="t1")
                nc.vector.tensor_tensor(t1[:], ps_bi[:], s_ai[:], AluOpType.mult)
                nc.vector.tensor_tensor(pr[:], pr[:], t1[:], AluOpType.add)
                pi = tpool.tile([B_SH, NF], F32, name="pi")
                nc.vector.tensor_tensor(pi[:], ps_br[:], s_ai[:], AluOpType.mult)
                t2 = tpool.tile([B_SH, NF], F32, name="t2")
                nc.vector.tensor_tensor(t2[:], ps_bi[:], s_ar[:], AluOpType.mult)
                nc.vector.tensor_tensor(pi[:], pi[:], t2[:], AluOpType.subtract)

                # G = R' . P  (w/d^2 already folded into R'), score = rowsum(G)
                g_t = tpool.tile([B_SH, 2 * NF], F32, name="g_t")
                nc.vector.tensor_tensor(g_t[:, 0:NF], ps_rr_f[:], pr[:],
                                        AluOpType.mult)
                nc.vector.tensor_tensor(g_t[:, NF:2 * NF], ps_ri_f[:], pi[:],
                                        AluOpType.mult)

            score = tpool.tile([B_SH, 1], F32, name="score")
            nc.vector.reduce_sum(score[:], g_t[:], axis=mybir.AxisListType.X)
            sig = tpool.tile([B_SH, 1], F32, name="sig")
            nc.scalar.activation(sig[:], score[:],
                                 mybir.ActivationFunctionType.Sigmoid)
            nc.sync.dma_start(out_d[:], sig[:])

    nc.compile()
    return nc


def _get_program():
    if "nc" not in _cached:
        _cached["nc"] = _build_program()
    return _cached["nc"]


def kernel(x, y, r, W_e, W_r):
    nc = _get_program()
    bf = ml_dtypes.bfloat16

    f_ab, f_r = _dft_bases()

    # relation-side tensors (shared / batch-sharded)
    wrT = np.zeros((R_PAD, D), dtype=bf)
    wrT[:R, :] = W_r.astype(bf).T
    rT_pad = np.zeros((R_PAD, B), dtype=bf)
    rT_pad[:R, :] = np.ascontiguousarray(r.T).astype(bf)

    xT = np.ascontiguousarray(x.T).astype(bf)     # (E, B)
    yT = np.ascontiguousarray(y.T).astype(bf)
    weT = np.ascontiguousarray(W_e.T).astype(bf)  # (E, D)

    in_maps = []
    for c in range(NCORES):
        lo, hi = c * E_SH, (c + 1) * E_SH
        xT_sh = np.zeros((E_PAD, B), dtype=bf)
        xT_sh[:E_SH] = xT[lo:hi]
        yT_sh = np.zeros((E_PAD, B), dtype=bf)
        yT_sh[:E_SH] = yT[lo:hi]
        weT_sh = np.zeros((E_PAD, D), dtype=bf)
        weT_sh[:E_SH] = weT[lo:hi]
        in_maps.append({
            "xT": xT_sh,
            "yT": yT_sh,
            "weT": weT_sh,
            "rT": np.ascontiguousarray(rT_pad[:, c * B_SH:(c + 1) * B_SH]),
            "wrT": wrT,
            "fab": f_ab,
            "fr": f_r,
        })

    res = run_bass_kernel_spmd(nc, in_maps, core_ids=list(range(NCORES)))
    out = np.concatenate([res.results[c]["out"] for c in range(NCORES)], axis=0)
    return out.astype(np.float32)


# revision 2
# speedup vs baseline: 1.0613x; 1.0613x over previous
"""HolE scorer kernel for 8 Trainium2 NeuronCores (Bass/Tile).

Computation (reference):
    a = x @ W_e.T; b = y @ W_e.T; rr = r @ W_r.T          # (B, d)
    corr = irfft(rfft(a) * conj(rfft(b))) / d             # circular correlation
    out = sigmoid(sum(rr * corr, axis=1))                 # (B, 1)

Strategy:
  - Tensor-parallel over entities for the two big GEMMs: core c holds
    entity columns [c*12500, (c+1)*12500) of x, y, W_e (padded to 12544 =
    98*128), computes partial a.T/b.T (d-major), ReduceScatter(add) over
    the 8 cores hands core c the fully-summed 128-batch-row slice.
  - Tail per core (128 batch rows): rr.T GEMM, rfft via DFT-basis matmuls,
    and the irfft+rowwise-dot folded into a frequency-domain weighted dot
    (Parseval):  score_i = (1/d^2) sum_f w_f (Rr*Pr + Ri*Pi)[i,f],
    with P = A * conj(B), w = [1, 2, ..., 2, 1].  The w/d^2 factor is
    folded into the DFT basis used for R, so score = reduce_sum(R' . P).
  - All matmul inputs in bf16 (fp32 PSUM accumulation): validated max rel
    err ~2e-3 on the final sigmoid output.
  - Queue split: weight/static/staging DMAs ride the Scalar HWDGE queue,
    the streamed x/y tiles ride the Sync queue, so big preloads don't
    head-of-line-block the stream.  y is processed before x so only the
    x reduce-scatter (plus a short a-FFT/DVE tail) is exposed at the end.
"""

import numpy as np
import ml_dtypes

import concourse.bass as bass
import concourse.tile as tile
from concourse import bacc, mybir
from concourse.alu_op_type import AluOpType
from concourse.bass_utils import run_bass_kernel_spmd

# Problem shapes (hardcoded per contract)
B = 1024            # batch
D = 512             # num_dim
E = 100000          # num_entities
R = 1000            # num_relations
NCORES = 8

E_SH = E // NCORES          # 12500 entities per core
KC = 98                     # k-chunks of 128 after padding (98*128 = 12544)
E_PAD = KC * 128            # 12544
KG = 7                      # k-groups
KJ = KC // KG               # 14 chunks per group
RC = 8                      # relation k-chunks (1000 -> 1024)
R_PAD = RC * 128
NF = D // 2 + 1             # 257 rfft bins
B_SH = B // NCORES          # 128 batch rows per core

BF16 = mybir.dt.bfloat16
F32 = mybir.dt.float32

_cached = {}


def _dft_bases():
    d = D
    dd = np.arange(d, dtype=np.float64)[:, None]
    ff = np.arange(NF, dtype=np.float64)[None, :]
    ang = 2.0 * np.pi * dd * ff / d
    fr = np.cos(ang)
    fi = -np.sin(ang)
    f_ab = np.concatenate([fr, fi], axis=1)              # (512, 514)
    w = np.full(NF, 2.0); w[0] = 1.0; w[-1] = 1.0
    scale = w / (d * d)
    f_r = np.concatenate([fr * scale, fi * scale], axis=1)
    return (f_ab.astype(ml_dtypes.bfloat16), f_r.astype(ml_dtypes.bfloat16))


def _build_program():
    nc = bacc.Bacc("TRN2", target_bir_lowering=False, debug=False,
                   num_devices=NCORES)

    xT_d = nc.dram_tensor("xT", (E_PAD, B), BF16, kind="ExternalInput")
    yT_d = nc.dram_tensor("yT", (E_PAD, B), BF16, kind="ExternalInput")
    weT_d = nc.dram_tensor("weT", (E_PAD, D), BF16, kind="ExternalInput")
    rT_d = nc.dram_tensor("rT", (R_PAD, B_SH), BF16, kind="ExternalInput")
    wrT_d = nc.dram_tensor("wrT", (R_PAD, D), BF16, kind="ExternalInput")
    fab_d = nc.dram_tensor("fab", (D, 2 * NF), BF16, kind="ExternalInput")
    fr_d = nc.dram_tensor("fr", (D, 2 * NF), BF16, kind="ExternalInput")
    out_d = nc.dram_tensor("out", (B_SH, 1), F32, kind="ExternalOutput")

    # (core, dim, batch) staging for the reduce-scatters
    stage_a = nc.dram_tensor("stage_a", (NCORES, D, B_SH), F32)
    stage_b = nc.dram_tensor("stage_b", (NCORES, D, B_SH), F32)
    rs_a = nc.dram_tensor("rs_a", (D, B_SH), F32)
    rs_b = nc.dram_tensor("rs_b", (D, B_SH), F32)
    groups = [list(range(NCORES))]

    with tile.TileContext(nc) as tc:
        with (
            tc.tile_pool(name="weights", bufs=1) as wpool,
            tc.tile_pool(name="stream", bufs=2) as spool,
            tc.tile_pool(name="copies", bufs=4) as cpool,
            tc.tile_pool(name="tail", bufs=1) as tpool,
            tc.tile_pool(name="psum", bufs=8, space="PSUM") as ppool,
        ):
            # ---- small static tensors first (Scalar queue) ----
            wr_t = wpool.tile([128, RC, D], BF16, tag="wr", name="wr")
            nc.scalar.dma_start(
                wr_t[:], wrT_d[:].rearrange("(j p) q -> p j q", p=128))
            r_t = wpool.tile([128, RC, B_SH], BF16, tag="r", name="r")
            nc.scalar.dma_start(
                r_t[:], rT_d[:].rearrange("(j p) q -> p j q", p=128))
            fab_t = wpool.tile([128, 4, 2 * NF], BF16, tag="fab", name="fab")
            nc.scalar.dma_start(
                fab_t[:], fab_d[:].rearrange("(j p) q -> p j q", p=128))
            fr_t = wpool.tile([128, 4, 2 * NF], BF16, tag="frq", name="frq")
            nc.scalar.dma_start(
                fr_t[:], fr_d[:].rearrange("(j p) q -> p j q", p=128))

            # ---- rr.T GEMM + its rfft while the big DMAs stream in ----
            ps_rr = ppool.tile([128, 4, B_SH], F32, tag="acc", name="ps_rr")
            for m in range(4):
                for j in range(RC):
                    nc.tensor.matmul(
                        ps_rr[:, m, :],
                        wr_t[:, j, m * 128:(m + 1) * 128],
                        r_t[:, j, :],
                        start=(j == 0), stop=(j == RC - 1))
            rr_b = tpool.tile([128, 4, B_SH], BF16, name="rr_b")
            nc.vector.tensor_copy(rr_b[:], ps_rr[:])

            ps_qr = ppool.tile([B_SH, NF], F32, tag="acc", name="ps_qr")
            ps_qi = ppool.tile([B_SH, NF], F32, tag="acc", name="ps_qi")
            for k in range(4):
                nc.tensor.matmul(ps_qr[:], rr_b[:, k, :], fr_t[:, k, 0:NF],
                                 start=(k == 0), stop=(k == 3))
            for k in range(4):
                nc.tensor.matmul(ps_qi[:], rr_b[:, k, :], fr_t[:, k, NF:2 * NF],
                                 start=(k == 0), stop=(k == 3))
            s_qr = tpool.tile([B_SH, NF], F32, name="s_qr")
            nc.vector.tensor_copy(s_qr[:], ps_qr[:])
            s_qi = tpool.tile([B_SH, NF], F32, name="s_qi")
            nc.vector.tensor_copy(s_qi[:], ps_qi[:])

            # ---- resident W_e.T groups (Scalar queue) ----
            we_tiles = []
            for g in range(KG):
                wt = wpool.tile([128, KJ, D], BF16, tag=f"we{g}", name=f"we{g}")
                nc.scalar.dma_start(
                    wt[:],
                    weT_d[g * KJ * 128:(g + 1) * KJ * 128, :]
                    .rearrange("(j p) q -> p j q", p=128))
                we_tiles.append(wt)

            # ---- main GEMMs: y first, then x ----
            for mi, (mat_d, stage, rs_out) in enumerate(
                    [(yT_d, stage_b, rs_b), (xT_d, stage_a, rs_a)]):
                for n in range(B // 512):
                    accs = [
                        ppool.tile([128, 512], F32, tag="acc",
                                   name=f"acc{mi}{n}{m}")
                        for m in range(4)
                    ]
                    for g in range(KG):
                        xt = spool.tile([128, KJ, 512], BF16, tag="xs",
                                        name=f"xs{mi}{n}{g}")
                        nc.sync.dma_start(
                            xt[:],
                            mat_d[g * KJ * 128:(g + 1) * KJ * 128,
                                  n * 512:(n + 1) * 512]
                            .rearrange("(j p) q -> p j q", p=128))
                        for j in range(KJ):
                            k = g * KJ + j
                            for m in range(4):
                                nc.tensor.matmul(
                                    accs[m][:],
                                    we_tiles[g][:, j, m * 128:(m + 1) * 128],
                                    xt[:, j, :],
                                    start=(k == 0), stop=(k == KC - 1))
                    for m in range(4):
                        sb = cpool.tile([128, 512], F32, tag="cp",
                                        name=f"cp{mi}{n}{m}")
                        nc.vector.tensor_copy(sb[:], accs[m][:])
                        dst = (stage[4 * n:4 * n + 4,
                                     m * 128:(m + 1) * 128, :]
                               .rearrange("c d q -> d c q"))
                        nc.scalar.dma_start(
                            dst, sb.rearrange("d (c q) -> d c q", c=4))
                # fire the reduce-scatter as soon as this matrix is done
                nc.gpsimd.collective_compute(
                    "ReduceScatter", AluOpType.add,
                    replica_groups=groups,
                    ins=[stage[:].opt()], outs=[rs_out[:].opt()])

            # ---- tail: rfft of b (ready mid-x), then a, then the score ----
            bT = tpool.tile([128, 4, B_SH], F32, name="bT")
            nc.scalar.dma_start(
                bT[:], rs_b[:].rearrange("(mc p) q -> p mc q", p=128))
            bT_b = tpool.tile([128, 4, B_SH], BF16, name="bT_b")
            nc.vector.tensor_copy(bT_b[:], bT[:])
            aT = tpool.tile([128, 4, B_SH], F32, name="aT")
            nc.scalar.dma_start(
                aT[:], rs_a[:].rearrange("(mc p) q -> p mc q", p=128))
            aT_b = tpool.tile([128, 4, B_SH], BF16, name="aT_b")
            nc.vector.tensor_copy(aT_b[:], aT[:])

            def rfft_mm(src_b, nm):
                psr = ppool.tile([B_SH, NF], F32, tag="acc", name=f"{nm}r")
                psi = ppool.tile([B_SH, NF], F32, tag="acc", name=f"{nm}i")
                for k in range(4):
                    nc.tensor.matmul(psr[:], src_b[:, k, :],
                                     fab_t[:, k, 0:NF],
                                     start=(k == 0), stop=(k == 3))
                for k in range(4):
                    nc.tensor.matmul(psi[:], src_b[:, k, :],
                                     fab_t[:, k, NF:2 * NF],
                                     start=(k == 0), stop=(k == 3))
                return psr, psi

            ps_br, ps_bi = rfft_mm(bT_b, "b")
            ps_ar, ps_ai = rfft_mm(aT_b, "a")

            s_ar = tpool.tile([B_SH, NF], F32, name="s_ar")
            nc.vector.tensor_copy(s_ar[:], ps_ar[:])
            s_ai = tpool.tile([B_SH, NF], F32, name="s_ai")
            nc.vector.tensor_copy(s_ai[:], ps_ai[:])

            # P = A * conj(B): Pr = ArBr + AiBi ; Pi = AiBr - ArBi
            pr = tpool.tile([B_SH, NF], F32, name="pr")
            nc.vector.tensor_tensor(pr[:], ps_br[:], s_ar[:], AluOpType.mult)
            t1 = tpool.tile([B_SH, NF], F32, name="t1")
            nc.vector.tensor_tensor(t1[:], ps_bi[:], s_ai[:], AluOpType.mult)
            nc.vector.tensor_tensor(pr[:], pr[:], t1[:], AluOpType.add)
            pi = tpool.tile([B_SH, NF], F32, name="pi")
            nc.vector.tensor_tensor(pi[:], ps_br[:], s_ai[:], AluOpType.mult)
            t2 = tpool.tile([B_SH, NF], F32, name="t2")
            nc.vector.tensor_tensor(t2[:], ps_bi[:], s_ar[:], AluOpType.mult)
            nc.vector.tensor_tensor(pi[:], pi[:], t2[:], AluOpType.subtract)

            # G = R' . P  (w/d^2 already folded into R'), score = rowsum(G)
            g_t = tpool.tile([B_SH, 2 * NF], F32, name="g_t")
            nc.vector.tensor_tensor(g_t[:, 0:NF], s_qr[:], pr[:],
                                    AluOpType.mult)
            nc.vector.tensor_tensor(g_t[:, NF:2 * NF], s_qi[:], pi[:],
                                    AluOpType.mult)

            score = tpool.tile([B_SH, 1], F32, name="score")
            nc.vector.reduce_sum(score[:], g_t[:], axis=mybir.AxisListType.X)
            sig = tpool.tile([B_SH, 1], F32, name="sig")
            nc.scalar.activation(sig[:], score[:],
                                 mybir.ActivationFunctionType.Sigmoid)
            nc.sync.dma_start(out_d[:], sig[:])

    nc.compile()
    return nc


def _get_program():
    if "nc" not in _cached:
        _cached["nc"] = _build_program()
    return _cached["nc"]


def kernel(x, y, r, W_e, W_r):
    nc = _get_program()
    bf = ml_dtypes.bfloat16

    f_ab, f_r = _dft_bases()

    # relation-side tensors (shared / batch-sharded)
    wrT = np.zeros((R_PAD, D), dtype=bf)
    wrT[:R, :] = W_r.astype(bf).T
    rT_pad = np.zeros((R_PAD, B), dtype=bf)
    rT_pad[:R, :] = np.ascontiguousarray(r.T).astype(bf)

    xT = np.ascontiguousarray(x.T).astype(bf)     # (E, B)
    yT = np.ascontiguousarray(y.T).astype(bf)
    weT = np.ascontiguousarray(W_e.T).astype(bf)  # (E, D)

    in_maps = []
    for c in range(NCORES):
        lo, hi = c * E_SH, (c + 1) * E_SH
        xT_sh = np.zeros((E_PAD, B), dtype=bf)
        xT_sh[:E_SH] = xT[lo:hi]
        yT_sh = np.zeros((E_PAD, B), dtype=bf)
        yT_sh[:E_SH] = yT[lo:hi]
        weT_sh = np.zeros((E_PAD, D), dtype=bf)
        weT_sh[:E_SH] = weT[lo:hi]
        in_maps.append({
            "xT": xT_sh,
            "yT": yT_sh,
            "weT": weT_sh,
            "rT": np.ascontiguousarray(rT_pad[:, c * B_SH:(c + 1) * B_SH]),
            "wrT": wrT,
            "fab": f_ab,
            "fr": f_r,
        })

    res = run_bass_kernel_spmd(nc, in_maps, core_ids=list(range(NCORES)))
    out = np.concatenate([res.results[c]["out"] for c in range(NCORES)], axis=0)
    return out.astype(np.float32)


# revision 5
# speedup vs baseline: 1.0866x; 1.0239x over previous
"""HolE scorer kernel for 8 Trainium2 NeuronCores (Bass/Tile).

Computation (reference):
    a = x @ W_e.T; b = y @ W_e.T; rr = r @ W_r.T          # (B, d)
    corr = irfft(rfft(a) * conj(rfft(b))) / d             # circular correlation
    out = sigmoid(sum(rr * corr, axis=1))                 # (B, 1)

Strategy:
  - Tensor-parallel over entities for the two big GEMMs: core c holds
    entity columns [c*12500, (c+1)*12500) of x, y, W_e (padded to 12544 =
    98*128), computing partial a.T/b.T (d-major).  Each 512-batch-column
    pass gets its own ReduceScatter(add): core c receives fully-summed
    batch columns {n*512 + c*64 .. +63} for both halves n=0,1 -> 128
    batch rows per core (interleaved mapping, host gathers accordingly).
  - Tail per core (128 batch rows): rr.T GEMM, rfft via DFT-basis matmuls,
    and the irfft+rowwise-dot folded into a frequency-domain weighted dot
    (Parseval):  score_i = (1/d^2) sum_f w_f (Rr*Pr + Ri*Pi)[i,f],
    with P = A * conj(B), w = [1, 2, ..., 2, 1].  The w/d^2 factor is
    folded into the DFT basis used for R, so score = reduce_sum(R' . P).
  - All matmul inputs in bf16 (fp32 PSUM accumulation): validated max rel
    err ~2e-3 on the final sigmoid output.
  - Queue split: W_e/static/staging DMAs ride the Scalar HWDGE queue, the
    streamed x/y tiles the Sync queue.  y is processed before x; the b-rfft
    and the first half of the a-rfft overlap the x passes, so only the
    last 1MB reduce-scatter plus a ~64-row tail is exposed at the end.
"""

import numpy as np
import ml_dtypes

import concourse.bass as bass
import concourse.tile as tile
from concourse import bacc, mybir
from concourse.alu_op_type import AluOpType
from concourse.bass_utils import run_bass_kernel_spmd

# Problem shapes (hardcoded per contract)
B = 1024            # batch
D = 512             # num_dim
E = 100000          # num_entities
R = 1000            # num_relations
NCORES = 8

E_SH = E // NCORES          # 12500 entities per core
KC = 98                     # k-chunks of 128 after padding (98*128 = 12544)
E_PAD = KC * 128            # 12544
KG = 7                      # k-groups
KJ = KC // KG               # 14 chunks per group
RC = 8                      # relation k-chunks (1000 -> 1024)
R_PAD = RC * 128
NF = D // 2 + 1             # 257 rfft bins
B_SH = B // NCORES          # 128 batch rows per core
CH = 64                     # batch columns handed to each core per pass

BF16 = mybir.dt.bfloat16
F32 = mybir.dt.float32

_cached = {}


def _dft_bases():
    d = D
    dd = np.arange(d, dtype=np.float64)[:, None]
    ff = np.arange(NF, dtype=np.float64)[None, :]
    ang = 2.0 * np.pi * dd * ff / d
    fr = np.cos(ang)
    fi = -np.sin(ang)
    f_ab = np.concatenate([fr, fi], axis=1)              # (512, 514)
    w = np.full(NF, 2.0); w[0] = 1.0; w[-1] = 1.0
    scale = w / (d * d)
    f_r = np.concatenate([fr * scale, fi * scale], axis=1)
    return (f_ab.astype(ml_dtypes.bfloat16), f_r.astype(ml_dtypes.bfloat16))


def _build_program():
    nc = bacc.Bacc("TRN2", target_bir_lowering=False, debug=False,
                   num_devices=NCORES)

    xT_d = nc.dram_tensor("xT", (E_PAD, B), BF16, kind="ExternalInput")
    yT_d = nc.dram_tensor("yT", (E_PAD, B), BF16, kind="ExternalInput")
    weT_d = nc.dram_tensor("weT", (E_PAD, D), BF16, kind="ExternalInput")
    rT_d = nc.dram_tensor("rT", (R_PAD, B_SH), BF16, kind="ExternalInput")
    wrT_d = nc.dram_tensor("wrT", (R_PAD, D), BF16, kind="ExternalInput")
    fab_d = nc.dram_tensor("fab", (D, 2 * NF), BF16, kind="ExternalInput")
    fr_d = nc.dram_tensor("fr", (D, 2 * NF), BF16, kind="ExternalInput")
    out_d = nc.dram_tensor("out", (B_SH, 1), F32, kind="ExternalOutput")

    # per-pass (core, dim, 64-batch) staging + reduce-scatter outputs
    stages = {}
    rs_outs = {}
    for mat in ("b", "a"):
        for n in range(2):
            stages[(mat, n)] = nc.dram_tensor(
                f"stage_{mat}{n}", (NCORES, D, CH), F32)
            rs_outs[(mat, n)] = nc.dram_tensor(
                f"rs_{mat}{n}", (D, CH), F32)
    groups = [list(range(NCORES))]

    with tile.TileContext(nc) as tc:
        with (
            tc.tile_pool(name="weights", bufs=1) as wpool,
            tc.tile_pool(name="stream", bufs=2) as spool,
            tc.tile_pool(name="copies", bufs=4) as cpool,
            tc.tile_pool(name="tail", bufs=1) as tpool,
            tc.tile_pool(name="psum", bufs=8, space="PSUM") as ppool,
        ):
            # ---- resident W_e.T groups (Scalar queue, we0 first) ----
            we_tiles = []
            for g in range(KG):
                wt = wpool.tile([128, KJ, D], BF16, tag=f"we{g}", name=f"we{g}")
                nc.scalar.dma_start(
                    wt[:],
                    weT_d[g * KJ * 128:(g + 1) * KJ * 128, :]
                    .rearrange("(j p) q -> p j q", p=128))
                we_tiles.append(wt)

            # small static tensors, needed only mid-kernel (Scalar queue)
            r_t = wpool.tile([128, RC, B_SH], BF16, tag="r", name="r")
            nc.scalar.dma_start(
                r_t[:], rT_d[:].rearrange("(j p) q -> p j q", p=128))
            wr_t = wpool.tile([128, RC, D], BF16, tag="wr", name="wr")
            nc.scalar.dma_start(
                wr_t[:], wrT_d[:].rearrange("(j p) q -> p j q", p=128))
            fab_t = wpool.tile([128, 4, 2 * NF], BF16, tag="fab", name="fab")
            nc.scalar.dma_start(
                fab_t[:], fab_d[:].rearrange("(j p) q -> p j q", p=128))
            fr_t = wpool.tile([128, 4, 2 * NF], BF16, tag="frq", name="frq")
            nc.scalar.dma_start(
                fr_t[:], fr_d[:].rearrange("(j p) q -> p j q", p=128))

            rr_b = tpool.tile([128, 4, B_SH], BF16, name="rr_b")

            # ---- main GEMMs: y first, then x; per-pass reduce-scatter ----
            for mi, (mat_d, mat) in enumerate([(yT_d, "b"), (xT_d, "a")]):
                for n in range(B // 512):
                    accs = [
                        ppool.tile([128, 512], F32, tag="acc",
                                   name=f"acc{mi}{n}{m}")
                        for m in range(4)
                    ]
                    for g in range(KG):
                        xt = spool.tile([128, KJ, 512], BF16, tag="xs",
                                        name=f"xs{mi}{n}{g}")
                        nc.sync.dma_start(
                            xt[:],
                            mat_d[g * KJ * 128:(g + 1) * KJ * 128,
                                  n * 512:(n + 1) * 512]
                            .rearrange("(j p) q -> p j q", p=128))
                        for j in range(KJ):
                            k = g * KJ + j
                            for m in range(4):
                                nc.tensor.matmul(
                                    accs[m][:],
                                    we_tiles[g][:, j, m * 128:(m + 1) * 128],
                                    xt[:, j, :],
                                    start=(k == 0), stop=(k == KC - 1))
                    for m in range(4):
                        sb = cpool.tile([128, 512], F32, tag="cp",
                                        name=f"cp{mi}{n}{m}")
                        nc.vector.tensor_copy(sb[:], accs[m][:])
                        dst = (stages[(mat, n)][:, m * 128:(m + 1) * 128, :]
                               .rearrange("c d q -> d c q"))
                        nc.scalar.dma_start(
                            dst, sb.rearrange("d (c q) -> d c q", c=NCORES))
                    nc.gpsimd.collective_compute(
                        "ReduceScatter", AluOpType.add,
                        replica_groups=groups,
                        ins=[stages[(mat, n)][:].opt()],
                        outs=[rs_outs[(mat, n)][:].opt()])

                    # rr.T GEMM slotted after the first pass: its inputs are
                    # small and arrive behind that pass's stream DMAs.
                    if mi == 0 and n == 0:
                        ps_rr = ppool.tile([128, 4, B_SH], F32, tag="acc",
                                           name="ps_rr")
                        for m in range(4):
                            for j in range(RC):
                                nc.tensor.matmul(
                                    ps_rr[:, m, :],
                                    wr_t[:, j, m * 128:(m + 1) * 128],
                                    r_t[:, j, :],
                                    start=(j == 0), stop=(j == RC - 1))
                        nc.vector.tensor_copy(rr_b[:], ps_rr[:])

            # ---- tail ----
            # b side: both halves ready once the y reduce-scatters finish
            # (early); a side: half 0 ready during x pass n=1, half 1 is the
            # only piece gated on the final reduce-scatter.
            def load_half(nm, mat, h):
                t = tpool.tile([128, 4, CH], F32, name=f"{nm}{h}")
                nc.scalar.dma_start(
                    t[:],
                    rs_outs[(mat, h)][:].rearrange("(mc p) q -> p mc q", p=128))
                tb = tpool.tile([128, 4, CH], BF16, name=f"{nm}b{h}")
                nc.vector.tensor_copy(tb[:], t[:])
                return tb

            def rfft_mm(src_b, basis, psr, psi, lo, w):
                for k in range(4):
                    nc.tensor.matmul(psr[lo:lo + w, :], src_b[:, k, :],
                                     basis[:, k, 0:NF],
                                     start=(k == 0), stop=(k == 3))
                for k in range(4):
                    nc.tensor.matmul(psi[lo:lo + w, :], src_b[:, k, :],
                                     basis[:, k, NF:2 * NF],
                                     start=(k == 0), stop=(k == 3))

            ps_br = ppool.tile([B_SH, NF], F32, tag="acc", name="ps_br")
            ps_bi = ppool.tile([B_SH, NF], F32, tag="acc", name="ps_bi")
            ps_qr = ppool.tile([B_SH, NF], F32, tag="acc", name="ps_qr")
            ps_qi = ppool.tile([B_SH, NF], F32, tag="acc", name="ps_qi")
            ps_ar = ppool.tile([B_SH, NF], F32, tag="acc", name="ps_ar")
            ps_ai = ppool.tile([B_SH, NF], F32, tag="acc", name="ps_ai")

            for h in range(2):
                b_half = load_half("bT", "b", h)
                rfft_mm(b_half, fab_t, ps_br, ps_bi, h * CH, CH)

            rfft_mm(rr_b, fr_t, ps_qr, ps_qi, 0, B_SH)   # full 128 rows

            s_br = tpool.tile([B_SH, NF], F32, name="s_br")
            nc.vector.tensor_copy(s_br[:], ps_br[:])
            s_bi = tpool.tile([B_SH, NF], F32, name="s_bi")
            nc.vector.tensor_copy(s_bi[:], ps_bi[:])

            g_t = tpool.tile([B_SH, 2 * NF], F32, name="g_t")
            pr = tpool.tile([B_SH, NF], F32, name="pr")
            pi = tpool.tile([B_SH, NF], F32, name="pi")
            t1 = tpool.tile([B_SH, NF], F32, name="t1")
            t2 = tpool.tile([B_SH, NF], F32, name="t2")

            for h in range(2):
                a_half = load_half("aT", "a", h)
                rfft_mm(a_half, fab_t, ps_ar, ps_ai, h * CH, CH)
                sl = slice(h * CH, (h + 1) * CH)
                # P = A * conj(B): Pr = ArBr + AiBi ; Pi = AiBr - ArBi
                nc.vector.tensor_tensor(pr[sl], ps_ar[sl], s_br[sl],
                                        AluOpType.mult)
                nc.vector.tensor_tensor(t1[sl], ps_ai[sl], s_bi[sl],
                                        AluOpType.mult)
                nc.vector.tensor_tensor(pr[sl], pr[sl], t1[sl], AluOpType.add)
                nc.vector.tensor_tensor(pi[sl], ps_ai[sl], s_br[sl],
                                        AluOpType.mult)
                nc.vector.tensor_tensor(t2[sl], ps_ar[sl], s_bi[sl],
                                        AluOpType.mult)
                nc.vector.tensor_tensor(pi[sl], pi[sl], t2[sl],
                                        AluOpType.subtract)
                # G = R' . P  (w/d^2 folded into R')
                nc.vector.tensor_tensor(g_t[sl, 0:NF], ps_qr[sl], pr[sl],
                                        AluOpType.mult)
                nc.vector.tensor_tensor(g_t[sl, NF:2 * NF], ps_qi[sl], pi[sl],
                                        AluOpType.mult)

            score = tpool.tile([B_SH, 1], F32, name="score")
            nc.vector.reduce_sum(score[:], g_t[:], axis=mybir.AxisListType.X)
            sig = tpool.tile([B_SH, 1], F32, name="sig")
            nc.scalar.activation(sig[:], score[:],
                                 mybir.ActivationFunctionType.Sigmoid)
            nc.sync.dma_start(out_d[:], sig[:])

    nc.compile()
    return nc


def _get_program():
    if "nc" not in _cached:
        _cached["nc"] = _build_program()
    return _cached["nc"]


def _core_rows(c):
    """Batch rows owned by core c: per-pass interleaved 64-row chunks."""
    return np.r_[c * CH:(c + 1) * CH, 512 + c * CH:512 + (c + 1) * CH]


def kernel(x, y, r, W_e, W_r):
    nc = _get_program()
    bf = ml_dtypes.bfloat16

    f_ab, f_r = _dft_bases()

    wrT = np.zeros((R_PAD, D), dtype=bf)
    wrT[:R, :] = W_r.astype(bf).T
    rT_pad = np.zeros((R_PAD, B), dtype=bf)
    rT_pad[:R, :] = np.ascontiguousarray(r.T).astype(bf)

    xT = np.ascontiguousarray(x.T).astype(bf)     # (E, B)
    yT = np.ascontiguousarray(y.T).astype(bf)
    weT = np.ascontiguousarray(W_e.T).astype(bf)  # (E, D)

    in_maps = []
    for c in range(NCORES):
        lo, hi = c * E_SH, (c + 1) * E_SH
        xT_sh = np.zeros((E_PAD, B), dtype=bf)
        xT_sh[:E_SH] = xT[lo:hi]
        yT_sh = np.zeros((E_PAD, B), dtype=bf)
        yT_sh[:E_SH] = yT[lo:hi]
        weT_sh = np.zeros((E_PAD, D), dtype=bf)
        weT_sh[:E_SH] = weT[lo:hi]
        in_maps.append({
            "xT": xT_sh,
            "yT": yT_sh,
            "weT": weT_sh,
            "rT": np.ascontiguousarray(rT_pad[:, _core_rows(c)]),
            "wrT": wrT,
            "fab": f_ab,
            "fr": f_r,
        })

    res = run_bass_kernel_spmd(nc, in_maps, core_ids=list(range(NCORES)))
    out = np.empty((B, 1), dtype=np.float32)
    for c in range(NCORES):
        out[_core_rows(c)] = res.results[c]["out"]
    return out


# revision 7
# speedup vs baseline: 1.1176x; 1.0285x over previous
"""HolE scorer kernel for 8 Trainium2 NeuronCores (Bass/Tile).

Computation (reference):
    a = x @ W_e.T; b = y @ W_e.T; rr = r @ W_r.T          # (B, d)
    corr = irfft(rfft(a) * conj(rfft(b))) / d             # circular correlation
    out = sigmoid(sum(rr * corr, axis=1))                 # (B, 1)

Strategy:
  - Tensor-parallel over entities for the two big GEMMs: core c holds
    entity columns [c*12500, (c+1)*12500) of x, y, W_e (padded to 12544 =
    98*128), computing partial a.T/b.T (d-major).  Each 512-batch-column
    pass gets its own ReduceScatter(add): core c receives fully-summed
    batch columns {n*512 + c*64 .. +63} for both halves n=0,1 -> 128
    batch rows per core (interleaved mapping, host gathers accordingly).
  - Tail per core (128 batch rows): rr.T GEMM, rfft via DFT-basis matmuls,
    and the irfft+rowwise-dot folded into a frequency-domain weighted dot
    (Parseval):  score_i = (1/d^2) sum_f w_f (Rr*Pr + Ri*Pi)[i,f],
    with P = A * conj(B), w = [1, 2, ..., 2, 1].  The w/d^2 factor is
    folded into the DFT basis used for R, so score = reduce_sum(R' . P).
  - All matmul inputs in bf16 (fp32 PSUM accumulation): validated max rel
    err ~2e-3 on the final sigmoid output.
  - Queue split: W_e/static/staging DMAs ride the Scalar HWDGE queue, the
    streamed x/y tiles the Sync queue.  y is processed before x; the b-rfft
    and the first half of the a-rfft overlap the x passes, so only the
    last 1MB reduce-scatter plus a ~64-row tail is exposed at the end.
"""

import numpy as np
import ml_dtypes

import concourse.bass as bass
import concourse.tile as tile
from concourse import bacc, mybir
from concourse.alu_op_type import AluOpType
from concourse.bass_utils import run_bass_kernel_spmd

# Problem shapes (hardcoded per contract)
B = 1024            # batch
D = 512             # num_dim
E = 100000          # num_entities
R = 1000            # num_relations
NCORES = 8

E_SH = E // NCORES          # 12500 entities per core
KC = 98                     # k-chunks of 128 after padding (98*128 = 12544)
E_PAD = KC * 128            # 12544
KG = 7                      # k-groups
KJ = KC // KG               # 14 chunks per group
RC = 8                      # relation k-chunks (1000 -> 1024)
R_PAD = RC * 128
NF = D // 2 + 1             # 257 rfft bins
B_SH = B // NCORES          # 128 batch rows per core
CH = 64                     # batch columns handed to each core per pass

BF16 = mybir.dt.bfloat16
F32 = mybir.dt.float32

_cached = {}


def _dft_bases():
    d = D
    dd = np.arange(d, dtype=np.float64)[:, None]
    ff = np.arange(NF, dtype=np.float64)[None, :]
    ang = 2.0 * np.pi * dd * ff / d
    fr = np.cos(ang)
    fi = -np.sin(ang)
    f_ab = np.concatenate([fr, fi], axis=1)              # (512, 514)
    w = np.full(NF, 2.0); w[0] = 1.0; w[-1] = 1.0
    scale = w / (d * d)
    f_r = np.concatenate([fr * scale, fi * scale], axis=1)
    return (f_ab.astype(ml_dtypes.bfloat16), f_r.astype(ml_dtypes.bfloat16))


def _build_program():
    nc = bacc.Bacc("TRN2", target_bir_lowering=False, debug=False,
                   num_devices=NCORES)

    xT_d = nc.dram_tensor("xT", (E_PAD, B), BF16, kind="ExternalInput")
    yT_d = nc.dram_tensor("yT", (E_PAD, B), BF16, kind="ExternalInput")
    weT_d = nc.dram_tensor("weT", (E_PAD, D), BF16, kind="ExternalInput")
    rT_d = nc.dram_tensor("rT", (R_PAD, B_SH), BF16, kind="ExternalInput")
    wrT_d = nc.dram_tensor("wrT", (R_PAD, D), BF16, kind="ExternalInput")
    fab_d = nc.dram_tensor("fab", (D, 2 * NF), BF16, kind="ExternalInput")
    fr_d = nc.dram_tensor("fr", (D, 2 * NF), BF16, kind="ExternalInput")
    out_d = nc.dram_tensor("out", (B_SH, 1), F32, kind="ExternalOutput")

    # per-pass (core, dim, 64-batch) staging + reduce-scatter outputs
    stages = {}
    rs_outs = {}
    for mat in ("b", "a"):
        for n in range(2):
            stages[(mat, n)] = nc.dram_tensor(
                f"stage_{mat}{n}", (NCORES, D, CH), F32)
            rs_outs[(mat, n)] = nc.dram_tensor(
                f"rs_{mat}{n}", (D, CH), F32)
    groups = [list(range(NCORES))]

    with tile.TileContext(nc) as tc:
        with (
            tc.tile_pool(name="weights", bufs=1) as wpool,
            tc.tile_pool(name="stream", bufs=3) as spool,
            tc.tile_pool(name="copies", bufs=4) as cpool,
            tc.tile_pool(name="tail", bufs=1) as tpool,
            tc.tile_pool(name="psum", bufs=8, space="PSUM") as ppool,
        ):
            # ---- resident W_e.T groups (Scalar queue, we0 first) ----
            we_tiles = []
            for g in range(KG):
                wt = wpool.tile([128, KJ, D], BF16, tag=f"we{g}", name=f"we{g}")
                src = (weT_d[g * KJ * 128:(g + 1) * KJ * 128, :]
                       .rearrange("(j p) q -> p j q", p=128))
                if g == 0:
                    half = KJ // 2
                    nc.scalar.dma_start(wt[:, :half], src[:, :half])
                    nc.scalar.dma_start(wt[:, half:], src[:, half:])
                else:
                    nc.scalar.dma_start(wt[:], src)
                we_tiles.append(wt)

            # small static tensors, needed only mid-kernel (Scalar queue)
            r_t = wpool.tile([128, RC, B_SH], BF16, tag="r", name="r")
            nc.scalar.dma_start(
                r_t[:], rT_d[:].rearrange("(j p) q -> p j q", p=128))
            wr_t = wpool.tile([128, RC, D], BF16, tag="wr", name="wr")
            nc.scalar.dma_start(
                wr_t[:], wrT_d[:].rearrange("(j p) q -> p j q", p=128))
            fab_t = wpool.tile([128, 4, 2 * NF], BF16, tag="fab", name="fab")
            nc.scalar.dma_start(
                fab_t[:], fab_d[:].rearrange("(j p) q -> p j q", p=128))
            fr_t = wpool.tile([128, 4, 2 * NF], BF16, tag="frq", name="frq")
            nc.scalar.dma_start(
                fr_t[:], fr_d[:].rearrange("(j p) q -> p j q", p=128))

            rr_b = tpool.tile([128, 4, B_SH], BF16, name="rr_b")

            # ---- main GEMMs: y first, then x; per-pass reduce-scatter ----
            def load_half(nm, mat, h):
                t = tpool.tile([128, 4, CH], F32, name=f"{nm}{h}")
                nc.scalar.dma_start(
                    t[:],
                    rs_outs[(mat, h)][:].rearrange("(mc p) q -> p mc q", p=128))
                tb = tpool.tile([128, 4, CH], BF16, name=f"{nm}b{h}")
                nc.vector.tensor_copy(tb[:], t[:])
                return tb

            def rfft_mm(src_b, basis, psr, psi, lo, w):
                for k in range(4):
                    nc.tensor.matmul(psr[lo:lo + w, :], src_b[:, k, :],
                                     basis[:, k, 0:NF],
                                     start=(k == 0), stop=(k == 3))
                for k in range(4):
                    nc.tensor.matmul(psi[lo:lo + w, :], src_b[:, k, :],
                                     basis[:, k, NF:2 * NF],
                                     start=(k == 0), stop=(k == 3))

            f1 = tpool.tile([B_SH, NF], F32, name="f1")
            f2 = tpool.tile([B_SH, NF], F32, name="f2")
            g_t = tpool.tile([B_SH, 2 * NF], F32, name="g_t")
            sig = tpool.tile([B_SH, 1], F32, name="sig")

            passes = [("b", yT_d, 0), ("b", yT_d, 1), ("a", xT_d, 0),
                      ("a", xT_d, 1)]
            for pi_, (mat, mat_d, n) in enumerate(passes):
                accs = [
                    ppool.tile([128, 512], F32, tag="acc",
                               name=f"acc{mat}{n}{m}")
                    for m in range(4)
                ]
                for g in range(KG):
                    xt = spool.tile([128, KJ, 512], BF16, tag="xs",
                                    name=f"xs{mat}{n}{g}")
                    src = (mat_d[g * KJ * 128:(g + 1) * KJ * 128,
                                 n * 512:(n + 1) * 512]
                           .rearrange("(j p) q -> p j q", p=128))
                    if pi_ == 0 and g == 0:
                        half = KJ // 2
                        nc.sync.dma_start(xt[:, :half], src[:, :half])
                        nc.sync.dma_start(xt[:, half:], src[:, half:])
                    else:
                        nc.sync.dma_start(xt[:], src)
                    for j in range(KJ):
                        k = g * KJ + j
                        for m in range(4):
                            nc.tensor.matmul(
                                accs[m][:],
                                we_tiles[g][:, j, m * 128:(m + 1) * 128],
                                xt[:, j, :],
                                start=(k == 0), stop=(k == KC - 1))
                for m in range(4):
                    sb = cpool.tile([128, 512], F32, tag="cp",
                                    name=f"cp{mat}{n}{m}")
                    nc.vector.tensor_copy(sb[:], accs[m][:])
                    dst = (stages[(mat, n)][:, m * 128:(m + 1) * 128, :]
                           .rearrange("c d q -> d c q"))
                    nc.scalar.dma_start(
                        dst, sb.rearrange("d (c q) -> d c q", c=NCORES))
                nc.gpsimd.collective_compute(
                    "ReduceScatter", AluOpType.add,
                    replica_groups=groups,
                    ins=[stages[(mat, n)][:].opt()],
                    outs=[rs_outs[(mat, n)][:].opt()])

                if pi_ == 0:
                    # rr.T GEMM slotted after the first pass: its inputs are
                    # small and arrive behind that pass's stream DMAs.
                    ps_rr = ppool.tile([128, 4, B_SH], F32, tag="acc",
                                       name="ps_rr")
                    for m in range(4):
                        for j in range(RC):
                            nc.tensor.matmul(
                                ps_rr[:, m, :],
                                wr_t[:, j, m * 128:(m + 1) * 128],
                                r_t[:, j, :],
                                start=(j == 0), stop=(j == RC - 1))
                    nc.vector.tensor_copy(rr_b[:], ps_rr[:])

                if pi_ == 2:
                    # b/rr rffts slotted between the two x passes: their
                    # reduce-scatters are long done, and the combined factors
                    # F1 = Rr.Br - Ri.Bi, F2 = Rr.Bi + Ri.Br free their PSUM
                    # banks before the last pass needs them.
                    ps_br = ppool.tile([B_SH, NF], F32, tag="acc", name="ps_br")
                    ps_bi = ppool.tile([B_SH, NF], F32, tag="acc", name="ps_bi")
                    ps_qr = ppool.tile([B_SH, NF], F32, tag="acc", name="ps_qr")
                    ps_qi = ppool.tile([B_SH, NF], F32, tag="acc", name="ps_qi")
                    for h in range(2):
                        b_half = load_half("bT", "b", h)
                        rfft_mm(b_half, fab_t, ps_br, ps_bi, h * CH, CH)
                    rfft_mm(rr_b, fr_t, ps_qr, ps_qi, 0, B_SH)
                    s_qr = tpool.tile([B_SH, NF], F32, name="s_qr")
                    nc.vector.tensor_copy(s_qr[:], ps_qr[:])
                    s_qi = tpool.tile([B_SH, NF], F32, name="s_qi")
                    nc.vector.tensor_copy(s_qi[:], ps_qi[:])
                    t1 = tpool.tile([B_SH, NF], F32, name="t1")
                    t2 = tpool.tile([B_SH, NF], F32, name="t2")
                    nc.vector.tensor_tensor(f1[:], ps_br[:], s_qr[:],
                                            AluOpType.mult)
                    nc.vector.tensor_tensor(t1[:], ps_bi[:], s_qi[:],
                                            AluOpType.mult)
                    nc.vector.tensor_tensor(f1[:], f1[:], t1[:],
                                            AluOpType.subtract)
                    nc.vector.tensor_tensor(f2[:], ps_bi[:], s_qr[:],
                                            AluOpType.mult)
                    nc.vector.tensor_tensor(t2[:], ps_br[:], s_qi[:],
                                            AluOpType.mult)
                    nc.vector.tensor_tensor(f2[:], f2[:], t2[:],
                                            AluOpType.add)

            # ---- tail: score_f = Ar.F1 + Ai.F2, rowsum, sigmoid ----
            ps_ar = ppool.tile([B_SH, NF], F32, tag="acc", name="ps_ar")
            ps_ai = ppool.tile([B_SH, NF], F32, tag="acc", name="ps_ai")
            for h in range(2):
                a_half = load_half("aT", "a", h)
                rfft_mm(a_half, fab_t, ps_ar, ps_ai, h * CH, CH)
                sl = slice(h * CH, (h + 1) * CH)
                nc.vector.tensor_tensor(g_t[sl, 0:NF], ps_ar[sl], f1[sl],
                                        AluOpType.mult)
                nc.vector.tensor_tensor(g_t[sl, NF:2 * NF], ps_ai[sl], f2[sl],
                                        AluOpType.mult)
                score = tpool.tile([CH, 1], F32, tag="score", name=f"score{h}")
                nc.vector.reduce_sum(score[:], g_t[sl, :],
                                     axis=mybir.AxisListType.X)
                nc.scalar.activation(sig[sl], score[:],
                                     mybir.ActivationFunctionType.Sigmoid)
                nc.sync.dma_start(out_d[h * CH:(h + 1) * CH, :], sig[sl])

    nc.compile()
    return nc


def _get_program():
    if "nc" not in _cached:
        _cached["nc"] = _build_program()
    return _cached["nc"]


def _core_rows(c):
    """Batch rows owned by core c: per-pass interleaved 64-row chunks."""
    return np.r_[c * CH:(c + 1) * CH, 512 + c * CH:512 + (c + 1) * CH]


def kernel(x, y, r, W_e, W_r):
    nc = _get_program()
    bf = ml_dtypes.bfloat16

    f_ab, f_r = _dft_bases()

    wrT = np.zeros((R_PAD, D), dtype=bf)
    wrT[:R, :] = W_r.astype(bf).T
    rT_pad = np.zeros((R_PAD, B), dtype=bf)
    rT_pad[:R, :] = np.ascontiguousarray(r.T).astype(bf)

    xT = np.ascontiguousarray(x.T).astype(bf)     # (E, B)
    yT = np.ascontiguousarray(y.T).astype(bf)
    weT = np.ascontiguousarray(W_e.T).astype(bf)  # (E, D)

    in_maps = []
    for c in range(NCORES):
        lo, hi = c * E_SH, (c + 1) * E_SH
        xT_sh = np.zeros((E_PAD, B), dtype=bf)
        xT_sh[:E_SH] = xT[lo:hi]
        yT_sh = np.zeros((E_PAD, B), dtype=bf)
        yT_sh[:E_SH] = yT[lo:hi]
        weT_sh = np.zeros((E_PAD, D), dtype=bf)
        weT_sh[:E_SH] = weT[lo:hi]
        in_maps.append({
            "xT": xT_sh,
            "yT": yT_sh,
            "weT": weT_sh,
            "rT": np.ascontiguousarray(rT_pad[:, _core_rows(c)]),
            "wrT": wrT,
            "fab": f_ab,
            "fr": f_r,
        })

    res = run_bass_kernel_spmd(nc, in_maps, core_ids=list(range(NCORES)))
    out = np.empty((B, 1), dtype=np.float32)
    for c in range(NCORES):
        out[_core_rows(c)] = res.results[c]["out"]
    return out


# revision 8
# speedup vs baseline: 1.1428x; 1.0226x over previous
"""HolE scorer kernel for 8 Trainium2 NeuronCores (Bass/Tile).

Computation (reference):
    a = x @ W_e.T; b = y @ W_e.T; rr = r @ W_r.T          # (B, d)
    corr = irfft(rfft(a) * conj(rfft(b))) / d             # circular correlation
    out = sigmoid(sum(rr * corr, axis=1))                 # (B, 1)

Strategy:
  - Tensor-parallel over entities for the two big GEMMs: core c holds
    entity columns [c*12500, (c+1)*12500) of x, y, W_e (padded to 12544 =
    98*128), computing partial a.T/b.T (d-major).  Each 512-batch-column
    pass gets its own ReduceScatter(add): core c receives fully-summed
    batch columns {n*512 + c*64 .. +63} for both halves n=0,1 -> 128
    batch rows per core (interleaved mapping, host gathers accordingly).
  - Tail per core (128 batch rows): rr.T GEMM, rfft via DFT-basis matmuls,
    and the irfft+rowwise-dot folded into a frequency-domain weighted dot
    (Parseval):  score_i = (1/d^2) sum_f w_f (Rr*Pr + Ri*Pi)[i,f],
    with P = A * conj(B), w = [1, 2, ..., 2, 1].  The w/d^2 factor is
    folded into the DFT basis used for R, so score = reduce_sum(R' . P).
  - All matmul inputs in bf16 (fp32 PSUM accumulation): validated max rel
    err ~2e-3 on the final sigmoid output.
  - Queue split: W_e/static/staging DMAs ride the Scalar HWDGE queue, the
    streamed x/y tiles the Sync queue.  y is processed before x; the b-rfft
    and the first half of the a-rfft overlap the x passes, so only the
    last 1MB reduce-scatter plus a ~64-row tail is exposed at the end.
"""

import numpy as np
import ml_dtypes

import concourse.bass as bass
import concourse.tile as tile
from concourse import bacc, mybir
from concourse.alu_op_type import AluOpType
from concourse.bass_utils import run_bass_kernel_spmd

# Problem shapes (hardcoded per contract)
B = 1024            # batch
D = 512             # num_dim
E = 100000          # num_entities
R = 1000            # num_relations
NCORES = 8

E_SH = E // NCORES          # 12500 entities per core
KC = 98                     # k-chunks of 128 after padding (98*128 = 12544)
E_PAD = KC * 128            # 12544
KG = 7                      # k-groups
KJ = KC // KG               # 14 chunks per group
RC = 8                      # relation k-chunks (1000 -> 1024)
R_PAD = RC * 128
NF = D // 2 + 1             # 257 rfft bins
B_SH = B // NCORES          # 128 batch rows per core
CH = 64                     # batch columns handed to each core per pass

BF16 = mybir.dt.bfloat16
F32 = mybir.dt.float32

_cached = {}


def _dft_bases():
    d = D
    dd = np.arange(d, dtype=np.float64)[:, None]
    ff = np.arange(NF, dtype=np.float64)[None, :]
    ang = 2.0 * np.pi * dd * ff / d
    fr = np.cos(ang)
    fi = -np.sin(ang)
    f_ab = np.concatenate([fr, fi], axis=1)              # (512, 514)
    w = np.full(NF, 2.0); w[0] = 1.0; w[-1] = 1.0
    scale = w / (d * d)
    f_r = np.concatenate([fr * scale, fi * scale], axis=1)
    return (f_ab.astype(ml_dtypes.bfloat16), f_r.astype(ml_dtypes.bfloat16))


def _build_program():
    nc = bacc.Bacc("TRN2", target_bir_lowering=False, debug=False,
                   num_devices=NCORES)

    xT_d = nc.dram_tensor("xT", (E_PAD, B), BF16, kind="ExternalInput")
    yT_d = nc.dram_tensor("yT", (E_PAD, B), BF16, kind="ExternalInput")
    weT_d = nc.dram_tensor("weT", (E_PAD, D), BF16, kind="ExternalInput")
    rT_d = nc.dram_tensor("rT", (R_PAD, B_SH), BF16, kind="ExternalInput")
    wrT_d = nc.dram_tensor("wrT", (R_PAD, D), BF16, kind="ExternalInput")
    fab_d = nc.dram_tensor("fab", (D, 2 * NF), BF16, kind="ExternalInput")
    fr_d = nc.dram_tensor("fr", (D, 2 * NF), BF16, kind="ExternalInput")
    out_d = nc.dram_tensor("out", (B_SH, 1), F32, kind="ExternalOutput")

    # per-pass (core, dim, 64-batch) staging + reduce-scatter outputs
    stages = {}
    rs_outs = {}
    for mat in ("b", "a"):
        for n in range(2):
            stages[(mat, n)] = nc.dram_tensor(
                f"stage_{mat}{n}", (NCORES, D, CH), BF16)
            rs_outs[(mat, n)] = nc.dram_tensor(
                f"rs_{mat}{n}", (D, CH), BF16)
    groups = [list(range(NCORES))]

    with tile.TileContext(nc) as tc:
        with (
            tc.tile_pool(name="weights", bufs=1) as wpool,
            tc.tile_pool(name="stream", bufs=3) as spool,
            tc.tile_pool(name="copies", bufs=4) as cpool,
            tc.tile_pool(name="tail", bufs=1) as tpool,
            tc.tile_pool(name="psum", bufs=8, space="PSUM") as ppool,
        ):
            # ---- resident W_e.T groups (Scalar queue, we0 first) ----
            we_tiles = []
            for g in range(KG):
                wt = wpool.tile([128, KJ, D], BF16, tag=f"we{g}", name=f"we{g}")
                src = (weT_d[g * KJ * 128:(g + 1) * KJ * 128, :]
                       .rearrange("(j p) q -> p j q", p=128))
                if g == 0:
                    half = KJ // 2
                    nc.scalar.dma_start(wt[:, :half], src[:, :half])
                    nc.scalar.dma_start(wt[:, half:], src[:, half:])
                else:
                    nc.scalar.dma_start(wt[:], src)
                we_tiles.append(wt)

            # small static tensors, needed only mid-kernel (Scalar queue)
            r_t = wpool.tile([128, RC, B_SH], BF16, tag="r", name="r")
            nc.scalar.dma_start(
                r_t[:], rT_d[:].rearrange("(j p) q -> p j q", p=128))
            wr_t = wpool.tile([128, RC, D], BF16, tag="wr", name="wr")
            nc.scalar.dma_start(
                wr_t[:], wrT_d[:].rearrange("(j p) q -> p j q", p=128))
            fab_t = wpool.tile([128, 4, 2 * NF], BF16, tag="fab", name="fab")
            nc.scalar.dma_start(
                fab_t[:], fab_d[:].rearrange("(j p) q -> p j q", p=128))
            fr_t = wpool.tile([128, 4, 2 * NF], BF16, tag="frq", name="frq")
            nc.scalar.dma_start(
                fr_t[:], fr_d[:].rearrange("(j p) q -> p j q", p=128))

            rr_b = tpool.tile([128, 4, B_SH], BF16, name="rr_b")

            # ---- main GEMMs: y first, then x; per-pass reduce-scatter ----
            def load_half(nm, mat, h):
                tb = tpool.tile([128, 4, CH], BF16, name=f"{nm}b{h}")
                nc.scalar.dma_start(
                    tb[:],
                    rs_outs[(mat, h)][:].rearrange("(mc p) q -> p mc q", p=128))
                return tb

            def rfft_mm(src_b, basis, psr, psi, lo, w):
                for k in range(4):
                    nc.tensor.matmul(psr[lo:lo + w, :], src_b[:, k, :],
                                     basis[:, k, 0:NF],
                                     start=(k == 0), stop=(k == 3))
                for k in range(4):
                    nc.tensor.matmul(psi[lo:lo + w, :], src_b[:, k, :],
                                     basis[:, k, NF:2 * NF],
                                     start=(k == 0), stop=(k == 3))

            f1 = tpool.tile([B_SH, NF], F32, name="f1")
            f2 = tpool.tile([B_SH, NF], F32, name="f2")
            g_t = tpool.tile([B_SH, 2 * NF], F32, name="g_t")
            sig = tpool.tile([B_SH, 1], F32, name="sig")

            passes = [("b", yT_d, 0), ("b", yT_d, 1), ("a", xT_d, 0),
                      ("a", xT_d, 1)]
            for pi_, (mat, mat_d, n) in enumerate(passes):
                accs = [
                    ppool.tile([128, 512], F32, tag="acc",
                               name=f"acc{mat}{n}{m}")
                    for m in range(4)
                ]
                for g in range(KG):
                    xt = spool.tile([128, KJ, 512], BF16, tag="xs",
                                    name=f"xs{mat}{n}{g}")
                    src = (mat_d[g * KJ * 128:(g + 1) * KJ * 128,
                                 n * 512:(n + 1) * 512]
                           .rearrange("(j p) q -> p j q", p=128))
                    if pi_ == 0 and g == 0:
                        half = KJ // 2
                        nc.sync.dma_start(xt[:, :half], src[:, :half])
                        nc.sync.dma_start(xt[:, half:], src[:, half:])
                    else:
                        nc.sync.dma_start(xt[:], src)
                    for j in range(KJ):
                        k = g * KJ + j
                        for m in range(4):
                            nc.tensor.matmul(
                                accs[m][:],
                                we_tiles[g][:, j, m * 128:(m + 1) * 128],
                                xt[:, j, :],
                                start=(k == 0), stop=(k == KC - 1))
                for m in range(4):
                    sb = cpool.tile([128, 512], BF16, tag="cp",
                                    name=f"cp{mat}{n}{m}")
                    nc.vector.tensor_copy(sb[:], accs[m][:])
                    dst = (stages[(mat, n)][:, m * 128:(m + 1) * 128, :]
                           .rearrange("c d q -> d c q"))
                    nc.scalar.dma_start(
                        dst, sb.rearrange("d (c q) -> d c q", c=NCORES))
                nc.gpsimd.collective_compute(
                    "ReduceScatter", AluOpType.add,
                    replica_groups=groups,
                    ins=[stages[(mat, n)][:].opt()],
                    outs=[rs_outs[(mat, n)][:].opt()])

                if pi_ == 0:
                    # rr.T GEMM slotted after the first pass: its inputs are
                    # small and arrive behind that pass's stream DMAs.
                    ps_rr = ppool.tile([128, 4, B_SH], F32, tag="acc",
                                       name="ps_rr")
                    for m in range(4):
                        for j in range(RC):
                            nc.tensor.matmul(
                                ps_rr[:, m, :],
                                wr_t[:, j, m * 128:(m + 1) * 128],
                                r_t[:, j, :],
                                start=(j == 0), stop=(j == RC - 1))
                    nc.vector.tensor_copy(rr_b[:], ps_rr[:])

                if pi_ == 2:
                    # b/rr rffts slotted between the two x passes: their
                    # reduce-scatters are long done, and the combined factors
                    # F1 = Rr.Br - Ri.Bi, F2 = Rr.Bi + Ri.Br free their PSUM
                    # banks before the last pass needs them.
                    ps_br = ppool.tile([B_SH, NF], F32, tag="acc", name="ps_br")
                    ps_bi = ppool.tile([B_SH, NF], F32, tag="acc", name="ps_bi")
                    ps_qr = ppool.tile([B_SH, NF], F32, tag="acc", name="ps_qr")
                    ps_qi = ppool.tile([B_SH, NF], F32, tag="acc", name="ps_qi")
                    for h in range(2):
                        b_half = load_half("bT", "b", h)
                        rfft_mm(b_half, fab_t, ps_br, ps_bi, h * CH, CH)
                    rfft_mm(rr_b, fr_t, ps_qr, ps_qi, 0, B_SH)
                    s_qr = tpool.tile([B_SH, NF], F32, name="s_qr")
                    nc.vector.tensor_copy(s_qr[:], ps_qr[:])
                    s_qi = tpool.tile([B_SH, NF], F32, name="s_qi")
                    nc.vector.tensor_copy(s_qi[:], ps_qi[:])
                    t1 = tpool.tile([B_SH, NF], F32, name="t1")
                    t2 = tpool.tile([B_SH, NF], F32, name="t2")
                    nc.vector.tensor_tensor(f1[:], ps_br[:], s_qr[:],
                                            AluOpType.mult)
                    nc.vector.tensor_tensor(t1[:], ps_bi[:], s_qi[:],
                                            AluOpType.mult)
                    nc.vector.tensor_tensor(f1[:], f1[:], t1[:],
                                            AluOpType.subtract)
                    nc.vector.tensor_tensor(f2[:], ps_bi[:], s_qr[:],
                                            AluOpType.mult)
                    nc.vector.tensor_tensor(t2[:], ps_br[:], s_qi[:],
                                            AluOpType.mult)
                    nc.vector.tensor_tensor(f2[:], f2[:], t2[:],
                                            AluOpType.add)

            # ---- tail: score_f = Ar.F1 + Ai.F2, rowsum, sigmoid ----
            ps_ar = ppool.tile([B_SH, NF], F32, tag="acc", name="ps_ar")
            ps_ai = ppool.tile([B_SH, NF], F32, tag="acc", name="ps_ai")
            for h in range(2):
                a_half = load_half("aT", "a", h)
                rfft_mm(a_half, fab_t, ps_ar, ps_ai, h * CH, CH)
                sl = slice(h * CH, (h + 1) * CH)
                nc.vector.tensor_tensor(g_t[sl, 0:NF], ps_ar[sl], f1[sl],
                                        AluOpType.mult)
                nc.vector.tensor_tensor(g_t[sl, NF:2 * NF], ps_ai[sl], f2[sl],
                                        AluOpType.mult)
                score = tpool.tile([CH, 1], F32, tag="score", name=f"score{h}")
                nc.vector.reduce_sum(score[:], g_t[sl, :],
                                     axis=mybir.AxisListType.X)
                nc.scalar.activation(sig[sl], score[:],
                                     mybir.ActivationFunctionType.Sigmoid)
                nc.sync.dma_start(out_d[h * CH:(h + 1) * CH, :], sig[sl])

    nc.compile()
    return nc


def _get_program():
    if "nc" not in _cached:
        _cached["nc"] = _build_program()
    return _cached["nc"]


def _core_rows(c):
    """Batch rows owned by core c: per-pass interleaved 64-row chunks."""
    return np.r_[c * CH:(c + 1) * CH, 512 + c * CH:512 + (c + 1) * CH]


def kernel(x, y, r, W_e, W_r):
    nc = _get_program()
    bf = ml_dtypes.bfloat16

    f_ab, f_r = _dft_bases()

    wrT = np.zeros((R_PAD, D), dtype=bf)
    wrT[:R, :] = W_r.astype(bf).T
    rT_pad = np.zeros((R_PAD, B), dtype=bf)
    rT_pad[:R, :] = np.ascontiguousarray(r.T).astype(bf)

    xT = np.ascontiguousarray(x.T).astype(bf)     # (E, B)
    yT = np.ascontiguousarray(y.T).astype(bf)
    weT = np.ascontiguousarray(W_e.T).astype(bf)  # (E, D)

    in_maps = []
    for c in range(NCORES):
        lo, hi = c * E_SH, (c + 1) * E_SH
        xT_sh = np.zeros((E_PAD, B), dtype=bf)
        xT_sh[:E_SH] = xT[lo:hi]
        yT_sh = np.zeros((E_PAD, B), dtype=bf)
        yT_sh[:E_SH] = yT[lo:hi]
        weT_sh = np.zeros((E_PAD, D), dtype=bf)
        weT_sh[:E_SH] = weT[lo:hi]
        in_maps.append({
            "xT": xT_sh,
            "yT": yT_sh,
            "weT": weT_sh,
            "rT": np.ascontiguousarray(rT_pad[:, _core_rows(c)]),
            "wrT": wrT,
            "fab": f_ab,
            "fr": f_r,
        })

    res = run_bass_kernel_spmd(nc, in_maps, core_ids=list(range(NCORES)))
    out = np.empty((B, 1), dtype=np.float32)
    for c in range(NCORES):
        out[_core_rows(c)] = res.results[c]["out"]
    return out
